# revision 74
# baseline (speedup 1.0000x reference)
"""Trainium2 Bass kernel for nn_ButterflyLayer1D.

Data-parallel across 8 NeuronCores: each core processes 128 of the 1024
samples; the butterfly filter tree is replicated to every core.

Per-core layout convention: activations live in SBUF as
(channels=128 partitions, free = [branch..., position..., sample(128)])
with samples innermost, so every matmul is a K=128 x M=128 weight applied
to 512-column tiles of the 8192-column activation plane.  All nine stages
(input conv, 3 down levels, middle switch, 3 up levels, output conv)
output exactly 8192 columns x 128 channels per core.

Matmuls run in bf16 (weights and activations; full-rate 1 col/cycle on the
PE array) with fp32 PSUM accumulation.  Per-branch biases are applied by
the Scalar/Vector engine epilogues (relu + bias from PSUM, greedy
load-balance between the two engines).  The middle switch has a distinct
bias per 128-col block: odd itx tiles seed it into PSUM via K=4 indicator
matmuls (+ Scalar relu), even tiles use a Vector TT-add with a broadcast
bias view (+ in-place DVE 4x relu), splitting the expensive mid epilogue
across engines.

Scheduling notes (from perfetto traces): input DMAs ride the two HWDGE
rings only (SWDGE starves them); xt streams in 1024-col chunks just-in-
time for an s0/L1 interleave ordered so the PE always has ready work
behind every epilogue wait; a few scratch warmup matmuls bridge the HAM
clock-gate (any >3.4us PE-idle hole drops the PE to 1.2 GHz); output
leaves in 2048-col DMAs with the final chunk split across both rings.
"""

import sys

for _p in ("/opt/trn_rl_repo",):
    if _p not in sys.path:
        sys.path.insert(0, _p)

import numpy as np
import ml_dtypes

import concourse.bass as bass
import concourse.bacc as bacc
import concourse.mybir as mybir
from concourse.tile import TileContext
from concourse.bass_utils import run_bass_kernel_spmd

C = 128            # channels == partitions == contraction size
N_CORES = 8
NPC = 128          # samples per core
NCOL = 64 * NPC    # 8192 free columns per stage
F32 = mybir.dt.float32
BF16 = mybir.dt.bfloat16
AF = mybir.ActivationFunctionType
ALU = mybir.AluOpType

PT = 1024          # epilogue sub-tile columns for per-branch-bias stages
PT2 = 2048         # psum tile columns (4 banks); 2 tiles fill PSUM
SUB = 512          # matmul moving-operand columns


def build_nc():
    nc = bacc.Bacc(enable_partition_id=False)

    dp = lambda name, shape, dt=BF16: nc.declare_dram_parameter(name, list(shape), dt, False)
    xt_d = dp("xt", (C, NCOL))
    wxf_d = dp("wxf", (C, C))
    w123_d = dp("w123", (C, 28 * C))      # [w1 | w2 | w3]
    wm_d = dp("wm", (C, 64 * C))
    w456k_d = dp("w456k", (C, 29 * C))    # [w4 | w5 | w6 | wkf]
    bia_d = dp("bia", (C, 93), F32)       # [xb|b1|b2|b3|b4|b5|b6|mb]
    mbi_d = dp("mbi", (4, 16 * C + 512))  # [mid-bias K=4 lhsT slices | indicator]
    out_d = nc.declare_dram_parameter("out", [C, NCOL], BF16, True)

    from contextlib import ExitStack

    with TileContext(nc) as tc, ExitStack() as ctx:
        singles = ctx.enter_context(tc.tile_pool(name="weights", bufs=1))
        act_pool = ctx.enter_context(tc.tile_pool(name="act", bufs=2))
        psum_pool = ctx.enter_context(tc.tile_pool(name="psum", bufs=4, space="PSUM"))

        def sb(shape, dt=BF16, name=None):
            return singles.tile(list(shape), dt, tag=name, name=name)

        # Critical-path loads first, spread across the three DMA issue rings
        # (sync + scalar HWDGE, gpsimd SWDGE) so the ~650ns per-DMA issue
        # cost doesn't serialize: sync carries the stage-0/level-1 critical
        # path (wxf, xt[0:512], w1), scalar and gpsimd split the bulk.
        xt = sb((C, NCOL), name="xt_sb")
        wxf = sb((C, C), name="wxf_sb")
        bia = sb((C, 93), dt=F32, name="bia_sb")
        w123 = sb((C, 28 * C), name="w123_sb")
        wm = sb((C, 64 * C), name="wm_sb")
        w456k = sb((C, 29 * C), name="w456k_sb")
        mbi = sb((4, 16 * C + 512), name="mbi_sb")
        mb2, ind = mbi[:, : 16 * C], mbi[:, 16 * C :]

        # PE warmup: scratch matmuls bridging to the first real matmul so
        # the HAM clock gate is warm when the stream starts.
        warm = sb((C, 512), name="warm_sb")
        nc.vector.memset(warm[:, :], 0.0)
        warm_ps = psum_pool.tile([C, PT], F32, tag="pt", name="pw")
        for _ in range(6):
            nc.tensor.matmul(warm_ps[:, 0:512], warm[:, 0:C], warm[:, :], start=True, stop=True)

        # Input DMAs ride the two HWDGE rings only (the gpsimd SWDGE ring
        # starves HWDGE under contention).  xt lands strictly first in
        # 1024-col chunks alternating rings (just-in-time for the s0/L1
        # interleave); bulk weights follow in need order.
        nc.sync.dma_start(out=wxf[:, :], in_=wxf_d[:, :])
        nc.scalar.dma_start(out=bia[:, :], in_=bia_d[:, :])
        nc.sync.dma_start(out=xt[:, 0:1024], in_=xt_d[:, 0:1024])
        nc.scalar.dma_start(out=xt[:, 1024:2048], in_=xt_d[:, 1024:2048])
        nc.sync.dma_start(out=w123[:, : 4 * C], in_=w123_d[:, : 4 * C])
        nc.scalar.dma_start(out=xt[:, 2048:3072], in_=xt_d[:, 2048:3072])
        nc.sync.dma_start(out=xt[:, 3072:4096], in_=xt_d[:, 3072:4096])
        nc.scalar.dma_start(out=xt[:, 4096:5120], in_=xt_d[:, 4096:5120])
        nc.sync.dma_start(out=xt[:, 5120:6144], in_=xt_d[:, 5120:6144])
        nc.scalar.dma_start(out=xt[:, 6144:7168], in_=xt_d[:, 6144:7168])
        nc.sync.dma_start(out=xt[:, 7168:8192], in_=xt_d[:, 7168:8192])
        # mbi/w3 issue on sync (idle sequencer, huge transfer slack) so the
        # Scalar sequencer frees up ~1.3us earlier for the ramp's epilogue
        # backlog; wm1 stays on scalar to keep ring transfer bytes balanced.
        nc.sync.dma_start(out=mbi[:, :], in_=mbi_d[:, :])
        nc.sync.dma_start(out=w123[:, 4 * C : 12 * C], in_=w123_d[:, 4 * C : 12 * C])
        nc.sync.dma_start(out=w123[:, 12 * C :], in_=w123_d[:, 12 * C :])
        nc.sync.dma_start(out=wm[:, : 32 * C], in_=wm_d[:, : 32 * C])
        nc.scalar.dma_start(out=wm[:, 32 * C :], in_=wm_d[:, 32 * C :])
        nc.sync.dma_start(out=w456k[:, :], in_=w456k_d[:, :])
        w1, w2, w3 = w123[:, : 4 * C], w123[:, 4 * C : 12 * C], w123[:, 12 * C : 28 * C]
        w4, w5 = w456k[:, : 16 * C], w456k[:, 16 * C : 24 * C]
        w6, wkf = w456k[:, 24 * C : 28 * C], w456k[:, 28 * C : 29 * C]
        xb, b1, b2 = bia[:, 0:1], bia[:, 1:3], bia[:, 3:7]
        b3, b4, b5 = bia[:, 7:15], bia[:, 15:23], bia[:, 23:27]
        b6, mb = bia[:, 27:29], bia[:, 29:93]

        load_ns = {"s": 3500.0, "v": 0.0}

        def epi(out_ap, in_ap, bias_ap, relu=True, cols=PT2):
            """One epilogue op: out = relu(in + bias) (or copy); greedy engine balance.
            V's fixed overhead is ~210ns measured (not the 120 uop model) --
            underestimating it overloads Vector by ~5us across the kernel."""
            cost = {"s": (352 + cols) / 1.2, "v": (210 + cols) / 0.96}
            eng = "s" if load_ns["s"] + cost["s"] <= load_ns["v"] + cost["v"] else "v"
            load_ns[eng] += cost[eng]
            if bias_ap is None and not relu:
                if eng == "s":
                    nc.scalar.activation(out_ap, in_ap, AF.Copy)
                else:
                    nc.vector.tensor_copy(out_ap, in_ap)
            elif bias_ap is None:
                if eng == "s":
                    nc.scalar.activation(out_ap, in_ap, AF.Relu)
                else:
                    nc.vector.tensor_scalar_max(out_ap, in_ap, 0.0)
            else:
                if eng == "s":
                    nc.scalar.activation(out_ap, in_ap, AF.Relu, bias=bias_ap)
                else:
                    nc.vector.tensor_scalar(out_ap, in_ap, bias_ap, 0.0, ALU.add, ALU.max)

        # ---------------- stage 0: input conv ----------------
        v0 = act_pool.tile([C, NCOL], BF16, tag="act", name="v0")

        def s0_tiles(bts, split=False):
            for bt in bts:
                for t in (2 * bt, 2 * bt + 1):
                    pt = psum_pool.tile([C, PT], F32, tag="pt", name="p0")
                    for s in range(2):
                        col = t * PT + s * SUB
                        nc.tensor.matmul(
                            pt[:, s * SUB : (s + 1) * SUB],
                            wxf[:, :],
                            xt[:, col : col + SUB],
                            start=True,
                            stop=True,
                        )
                    if split:
                        # halve the epilogue latency on chain-critical tiles
                        epi(v0[:, t * PT : t * PT + SUB], pt[:, 0:SUB], xb[:, 0:1], cols=SUB)
                        epi(v0[:, t * PT + SUB : (t + 1) * PT], pt[:, SUB:], xb[:, 0:1], cols=SUB)
                    else:
                        epi(v0[:, t * PT : (t + 1) * PT], pt[:, :], xb[:, 0:1], cols=PT)

        # ---------------- down levels 1..3 ----------------
        def down_level(vin, vout, w_sb, b_sb, nb_out, l_out, bts=None, split=()):
            """vin: (c, [nb_in, 2*l_out, n]); vout: (c, [nb_out, l_out, n]).
            Per 2048-col psum tile: matmuls grouped per branch (k outer, s
            inner -> same-weight runs); one 2048 epilogue when a single
            branch covers the tile, else two 1024 halves (per-branch bias)."""
            wv = w_sb.rearrange("p (b k d) -> p b k d", b=nb_out, k=2, d=C)
            vi = vin.rearrange("p (b l k n) -> p b l k n", b=nb_out // 2, l=l_out, k=2, n=NPC)
            vo = vout.rearrange("p (b l n) -> p b l n", b=nb_out, l=l_out, n=NPC)
            cpb = l_out * NPC  # columns per output branch (>= 1024)
            for bt in bts if bts is not None else range(NCOL // PT2):
                for t in (2 * bt, 2 * bt + 1):
                    pt = psum_pool.tile([C, PT], F32, tag="pt", name="pd")
                    for k in range(2):
                        for s in range(2):
                            col = t * PT + s * SUB
                            b = col // cpb
                            l0 = (col % cpb) // NPC
                            nc.tensor.matmul(
                                pt[:, s * SUB : (s + 1) * SUB],
                                wv[:, b, k, :],
                                vi[:, b // 2, l0 : l0 + SUB // NPC, k, :],
                                start=(k == 0),
                                stop=(k == 1),
                            )
                    b = (t * PT) // cpb
                    l0 = ((t * PT) % cpb) // NPC
                    if bt in split:
                        h = SUB // NPC
                        epi(vo[:, b, l0 : l0 + h, :], pt[:, 0:SUB], b_sb[:, b : b + 1], cols=SUB)
                        epi(vo[:, b, l0 + h : l0 + 2 * h, :], pt[:, SUB:], b_sb[:, b : b + 1], cols=SUB)
                    else:
                        epi(
                            vo[:, b, l0 : l0 + PT // NPC, :],
                            pt[:, :],
                            b_sb[:, b : b + 1],
                            cols=PT,
                        )

        v1 = act_pool.tile([C, NCOL], BF16, tag="act", name="v1")
        # Interleave s0 and L1 so the PE always has ready work queued behind
        # every epilogue wait: L1 bts (0,2) depend only on s0 (0,1), so L1(2)
        # fills the gap while s0(3)'s epilogue completes, and L1(3) covers
        # L1(1)'s epilogue latency before L2 starts.
        s0_tiles((0, 1))
        down_level(v0, v1, w1, b1, 2, 32, bts=(0,))
        s0_tiles((2,))
        s0_tiles((3,), split=True)
        down_level(v0, v1, w1, b1, 2, 32, bts=(2, 1, 3), split=(1,))
        v2 = act_pool.tile([C, NCOL], BF16, tag="act", name="v2")
        down_level(v1, v2, w2, b2, 4, 16)
        v3 = act_pool.tile([C, NCOL], BF16, tag="act", name="v3")
        # Every mid tile reads strided columns from ALL of v3, so the mid
        # stage gates on L3's last epilogue: split it across both engines.
        down_level(v2, v3, w3, b3, 8, 8, split=(3,))

        # ---------------- middle switch ----------------
        # v3: (c, [itk=8, itx=8, n]); vm: (c, [itx=8, itk=8, n])
        # The mid epilogue is the expensive one (per-block biases).  Split it:
        # odd tiles seed the bias into PSUM via K=4 indicator matmuls (cheap
        # on the PE) + a Scalar relu; even tiles use a Vector TT-add with a
        # broadcast bias view + an in-place DVE 4x relu.  This keeps both
        # engines fed without Vector becoming the mid-stage bottleneck.
        vm = act_pool.tile([C, NCOL], BF16, tag="act", name="vm")
        v3v = v3.rearrange("p (k x n) -> p k x n", k=8, x=8, n=NPC)
        wmv = wm.rearrange("p (k x d) -> p k x d", k=8, x=8, d=C)
        # Odd (indicator) tiles first in each pair: their bias matmuls don't
        # read v3, so they cover L3's final epilogue latency.
        def mid_tiles(ts):
          for t in ts:  # tile t covers itx = t
            pt = psum_pool.tile([C, PT], F32, tag="pt", name="pm")
            if t % 2 == 1:
                for sgrp in range(2):
                    nc.tensor.matmul(
                        pt[:, sgrp * SUB : (sgrp + 1) * SUB],
                        mb2[:, (2 * t + sgrp) * C : (2 * t + sgrp + 1) * C],
                        ind[:, :],
                        start=True,
                        stop=False,
                        skip_group_check=True,
                    )
                    for bi in range(4):
                        blk = 4 * sgrp + bi
                        nc.tensor.matmul(
                            pt[:, blk * NPC : (blk + 1) * NPC],
                            wmv[:, blk, t, :],
                            v3v[:, blk, t, :],
                            start=False,
                            stop=(bi == 3),
                            skip_group_check=True,
                        )
                nc.scalar.activation(
                    vm[:, t * PT : (t + 1) * PT], pt[:, :], AF.Relu
                )
                load_ns["s"] += (352 + PT) / 1.2
            else:
                for blk in range(8):  # block within tile (= itk)
                    nc.tensor.matmul(
                        pt[:, blk * NPC : (blk + 1) * NPC],
                        wmv[:, blk, t, :],
                        v3v[:, blk, t, :],
                        start=True,
                        stop=True,
                    )
                ptv = pt.rearrange("p (b n) -> p b n", b=8, n=NPC)
                bias_v = mb[:, 8 * t : 8 * (t + 1)].unsqueeze(2).broadcast_to((C, 8, NPC))
                dst = vm[:, t * PT : (t + 1) * PT]
                dstv = dst.rearrange("p (b n) -> p b n", b=8, n=NPC)
                nc.vector.tensor_tensor(dstv, ptv, bias_v, ALU.add)
                # in-place relu: bf16 SBUF contiguous -> DVE 4x mode, cheap
                nc.vector.tensor_scalar_max(dst, dst, 0.0)
                load_ns["v"] += (210 + PT) / 0.96 + (210 + PT / 4) / 0.96

        # ---------------- up levels 4..6 ----------------
        def up_level(vin, vout, w_sb, b_sb, nb_in, l_in, bts=None):
            """vin: (c, [x=nb_in, l_in, n]); vout: (c, [xo=nb_in/2, 2*l_in, n]);
            vout[:, xo, 2*l+j, :] = relu(sum_k vin[:, 2xo+k, l, :] @ W[xo,j,k] + B[xo,j])."""
            nbo = nb_in // 2
            wv = w_sb.rearrange("p (x j k d) -> p x j k d", x=nbo, j=2, k=2, d=C)
            vi = vin.rearrange("p (x l n) -> p x l n", x=nb_in, l=l_in, n=NPC)
            vo = vout.rearrange("p (x l j n) -> p x l j n", x=nbo, l=l_in, j=2, n=NPC)
            cpb = l_in * NPC  # columns per (xo, j) output block
            for bt in bts if bts is not None else range(NCOL // PT2):
                for t in (2 * bt, 2 * bt + 1):
                    pt = psum_pool.tile([C, PT], F32, tag="pt", name="pu")
                    for k in range(2):
                        for s in range(2):
                            col = t * PT + s * SUB
                            g = col // cpb  # (xo, j) block index, j-minor
                            xo, j = g // 2, g % 2
                            lt0 = (col % cpb) // NPC
                            nc.tensor.matmul(
                                pt[:, s * SUB : (s + 1) * SUB],
                                wv[:, xo, j, k, :],
                                vi[:, 2 * xo + k, lt0 : lt0 + SUB // NPC, :],
                                start=(k == 0),
                                stop=(k == 1),
                            )
                    g = (t * PT) // cpb
                    xo, j = g // 2, g % 2
                    lt0 = ((t * PT) % cpb) // NPC
                    epi(
                        vo[:, xo, lt0 : lt0 + PT // NPC, j, :],
                        pt[:, :],
                        b_sb[:, 2 * xo + j : 2 * xo + j + 1],
                        cols=PT,
                    )

        v4 = act_pool.tile([C, NCOL], BF16, tag="act", name="v4")
        mid_tiles((1, 0, 3, 2, 5, 4, 7, 6))
        up_level(vm, v4, w4, b4, 8, 8)
        v5 = act_pool.tile([C, NCOL], BF16, tag="act", name="v5")
        # L6 bt0/bt2 read v5 bts (0,2); emit those first so L6 can start
        # one L5 big-tile earlier.
        up_level(v4, v5, w5, b5, 4, 16, bts=(0, 2, 1, 3))
        v6 = act_pool.tile([C, NCOL], BF16, tag="act", name="v6")
        yo = singles.tile([C, NCOL], BF16, tag="yo_sb", name="yo")

        # ---------------- output conv (no bias / relu), interleaved with L6 --
        # Output leaves in 2048-col (512KB) DMAs alternating sync/scalar:
        # big enough for decent DMA bandwidth (4KB runs), small enough that
        # the final drain after the last epilogue stays short.
        out_rings = {0: nc.sync, 1: nc.scalar, 2: nc.sync, 3: nc.scalar}

        def out_tiles(bts):
            for bt in bts:
                for t in (2 * bt, 2 * bt + 1):
                    pt = psum_pool.tile([C, PT], F32, tag="pt", name="po")
                    for s in range(2):
                        col = t * PT + s * SUB
                        nc.tensor.matmul(
                            pt[:, s * SUB : (s + 1) * SUB],
                            wkf[:, :],
                            v6[:, col : col + SUB],
                            start=True,
                            stop=True,
                        )
                    if bt == 3:
                        # final tiles: 512-col epilogues on both engines so
                        # the last yo columns are ready sooner, then drain
                        # each 1024 immediately on its own ring.
                        epi(yo[:, t * PT : t * PT + SUB], pt[:, 0:SUB], None, relu=False, cols=SUB)
                        epi(yo[:, t * PT + SUB : (t + 1) * PT], pt[:, SUB:], None, relu=False, cols=SUB)
                        ring = nc.sync if t % 2 == 0 else nc.scalar
                        ring.dma_start(
                            out=out_d[:, t * PT : (t + 1) * PT],
                            in_=yo[:, t * PT : (t + 1) * PT],
                        )
                    else:
                        epi(yo[:, t * PT : (t + 1) * PT], pt[:, :], None, relu=False, cols=PT)
                if bt != 3:
                    out_rings[bt].dma_start(
                        out=out_d[:, bt * PT2 : (bt + 1) * PT2],
                        in_=yo[:, bt * PT2 : (bt + 1) * PT2],
                    )

        # L6 j=0 big-tiles are 0,1; j=1 are 2,3 (cpb=4096).  out big-tiles
        # (0,1) need L6 (0,2); out (2,3) need L6 (1,3): order so the PE has
        # L6 work queued behind every epilogue the out matmuls wait on.
        up_level(v5, v6, w6, b6, 2, 32, bts=(0, 2, 1))
        out_tiles((0,))
        up_level(v5, v6, w6, b6, 2, 32, bts=(3,))
        out_tiles((1, 2, 3))

    nc.finalize()
    return nc


_NC_CACHE = {}


def _get_nc():
    if "nc" not in _NC_CACHE:
        _NC_CACHE["nc"] = build_nc()
    return _NC_CACHE["nc"]


def _prep_in_maps(inputs):
    x = np.asarray(inputs["x"], np.float32)
    bf = lambda a: np.ascontiguousarray(np.asarray(a, np.float32)).astype(ml_dtypes.bfloat16)
    f32 = lambda a: np.ascontiguousarray(np.asarray(a, np.float32))
    mbv = np.asarray(inputs["mb"], np.float32)  # (k=8, x=8, c)
    mbT = mbv.transpose(1, 0, 2).reshape(64, C).T  # (c, 64), col = x*8 + k
    wmat = lambda key, nb: np.asarray(inputs[key], np.float32).reshape(nb, C, C).transpose(1, 0, 2).reshape(C, nb * C)
    w123 = np.concatenate([wmat("f1", 4), wmat("f2", 8), wmat("f3", 16)], axis=1)
    w456k = np.concatenate(
        [wmat("f4", 16), wmat("f5", 8), wmat("f6", 4), np.asarray(inputs["kf"], np.float32)], axis=1
    )
    bia = np.concatenate(
        [
            np.asarray(inputs["xb"], np.float32).reshape(C, 1),
            np.asarray(inputs["b1"], np.float32).T,
            np.asarray(inputs["b2"], np.float32).T,
            np.asarray(inputs["b3"], np.float32).T,
            np.asarray(inputs["b4"], np.float32).T,
            np.asarray(inputs["b5"], np.float32).T,
            np.asarray(inputs["b6"], np.float32).T,
            mbT,
        ],
        axis=1,
    )
    # mid-bias lhsT slices: u = 2*t + sgrp (t = itx tile, sgrp = 512-col half);
    # row ki covers block k = 4*sgrp + ki at x = t: mb2[ki, u*C+d] = mb[4*(u%2)+ki, u//2, d]
    mbi = np.zeros((4, 16 * C + 512), np.float32)
    for u in range(16):
        t_, sgrp = u // 2, u % 2
        for ki in range(4):
            mbi[ki, u * C : (u + 1) * C] = mbv[4 * sgrp + ki, t_, :]
    for ki in range(4):
        mbi[ki, 16 * C + ki * NPC : 16 * C + (ki + 1) * NPC] = 1.0
    shared = {
        "mbi": bf(mbi),
        "wxf": bf(inputs["xf"]),  # (f=128, c) as lhsT directly
        "w123": bf(w123),
        "wm": bf(np.asarray(inputs["md"], np.float32).reshape(64, C, C).transpose(1, 0, 2).reshape(C, 64 * C)),
        "w456k": bf(w456k),
        "bia": f32(bia),
    }
    in_maps = []
    for i in range(N_CORES):
        xs = x[i * NPC : (i + 1) * NPC]  # (128, 8192)
        xt = (
            np.ascontiguousarray(xs.reshape(NPC, 64, C).transpose(2, 1, 0))
            .reshape(C, NCOL)
            .astype(ml_dtypes.bfloat16)
        )
        in_maps.append({"xt": xt, **shared})
    return in_maps


def _gather(results):
    outs = []
    for i in range(N_CORES):
        r = np.asarray(results[i]["out"]).astype(np.float32)  # (C=k_out, [l=64, n=128])
        outs.append(r.reshape(C, 64, NPC).transpose(2, 1, 0).reshape(NPC, 64 * C))
    return np.concatenate(outs, axis=0).astype(np.float32)


def _enable_ntff_hook():
    """Register the axon NTFF profiling hook (missing from this image's
    antenv) so run_bass_kernel_spmd(trace=True) can measure HW exec time."""
    import types

    if "antenv.axon_hooks" in sys.modules:
        return
    import antenv
    from trn_agent_boot.trn_boot import _ntff_profile_via_ctypes

    hook = _ntff_profile_via_ctypes("/opt/axon/libaxon_pjrt.so")
    mod = types.ModuleType("antenv.axon_hooks")
    mod.get_axon_ntff_profile_hook = lambda: hook
    mod.set_axon_ntff_profile_hook = lambda h: None
    sys.modules["antenv.axon_hooks"] = mod
    antenv.axon_hooks = mod
    import concourse.bass_utils as bu

    bu.upload_artifacts = lambda tmpdir: tmpdir  # keep artifacts local


def run(inputs, trace=False, **kw):
    nc = _get_nc()
    in_maps = _prep_in_maps(inputs)
    if trace:
        _enable_ntff_hook()
    res = run_bass_kernel_spmd(nc, in_maps, core_ids=list(range(N_CORES)), trace=trace, **kw)
    return _gather(res.results), res


def kernel(**inputs) -> np.ndarray:
    out, _ = run(inputs, trace=False)
    return out



# revision 75
# speedup vs baseline: 1.0085x; 1.0085x over previous
"""Trainium2 Bass kernel for nn_ButterflyLayer1D.

Data-parallel across 8 NeuronCores: each core processes 128 of the 1024
samples; the butterfly filter tree is replicated to every core.

Per-core layout convention: activations live in SBUF as
(channels=128 partitions, free = [branch..., position..., sample(128)])
with samples innermost, so every matmul is a K=128 x M=128 weight applied
to 512-column tiles of the 8192-column activation plane.  All nine stages
(input conv, 3 down levels, middle switch, 3 up levels, output conv)
output exactly 8192 columns x 128 channels per core.

Matmuls run in bf16 (weights and activations; full-rate 1 col/cycle on the
PE array) with fp32 PSUM accumulation.  Per-branch biases are applied by
the Scalar/Vector engine epilogues (relu + bias from PSUM, greedy
load-balance between the two engines).  The middle switch has a distinct
bias per 128-col block: odd itx tiles seed it into PSUM via K=4 indicator
matmuls (+ Scalar relu), even tiles use a Vector TT-add with a broadcast
bias view (+ in-place DVE 4x relu), splitting the expensive mid epilogue
across engines.

Scheduling notes (from perfetto traces): input DMAs ride the two HWDGE
rings only (SWDGE starves them); xt streams in 1024-col chunks just-in-
time for an s0/L1 interleave ordered so the PE always has ready work
behind every epilogue wait; a few scratch warmup matmuls bridge the HAM
clock-gate (any >3.4us PE-idle hole drops the PE to 1.2 GHz); output
leaves in 2048-col DMAs with the final chunk split across both rings.
"""

import sys

for _p in ("/opt/trn_rl_repo",):
    if _p not in sys.path:
        sys.path.insert(0, _p)

import numpy as np
import ml_dtypes

import concourse.bass as bass
import concourse.bacc as bacc
import concourse.mybir as mybir
from concourse.tile import TileContext
from concourse.bass_utils import run_bass_kernel_spmd

C = 128            # channels == partitions == contraction size
N_CORES = 8
NPC = 128          # samples per core
NCOL = 64 * NPC    # 8192 free columns per stage
F32 = mybir.dt.float32
BF16 = mybir.dt.bfloat16
AF = mybir.ActivationFunctionType
ALU = mybir.AluOpType

PT = 1024          # epilogue sub-tile columns for per-branch-bias stages
PT2 = 2048         # psum tile columns (4 banks); 2 tiles fill PSUM
SUB = 512          # matmul moving-operand columns


def build_nc():
    nc = bacc.Bacc(enable_partition_id=False)

    dp = lambda name, shape, dt=BF16: nc.declare_dram_parameter(name, list(shape), dt, False)
    xt_d = dp("xt", (C, NCOL))
    wxf_d = dp("wxf", (C, C))
    w123_d = dp("w123", (C, 28 * C))      # [w1 | w2 | w3]
    wm_d = dp("wm", (C, 64 * C))
    w456k_d = dp("w456k", (C, 29 * C))    # [w4 | w5 | w6 | wkf]
    bia_d = dp("bia", (C, 93), F32)       # [xb|b1|b2|b3|b4|b5|b6|mb]
    mbi_d = dp("mbi", (4, 16 * C + 512))  # [mid-bias K=4 lhsT slices | indicator]
    out_d = nc.declare_dram_parameter("out", [C, NCOL], BF16, True)

    from contextlib import ExitStack

    with TileContext(nc) as tc, ExitStack() as ctx:
        singles = ctx.enter_context(tc.tile_pool(name="weights", bufs=1))
        act_pool = ctx.enter_context(tc.tile_pool(name="act", bufs=2))
        psum_pool = ctx.enter_context(tc.tile_pool(name="psum", bufs=4, space="PSUM"))

        def sb(shape, dt=BF16, name=None):
            return singles.tile(list(shape), dt, tag=name, name=name)

        # Critical-path loads first, spread across the three DMA issue rings
        # (sync + scalar HWDGE, gpsimd SWDGE) so the ~650ns per-DMA issue
        # cost doesn't serialize: sync carries the stage-0/level-1 critical
        # path (wxf, xt[0:512], w1), scalar and gpsimd split the bulk.
        xt = sb((C, NCOL), name="xt_sb")
        wxf = sb((C, C), name="wxf_sb")
        bia = sb((C, 93), dt=F32, name="bia_sb")
        w123 = sb((C, 28 * C), name="w123_sb")
        wm = sb((C, 64 * C), name="wm_sb")
        w456k = sb((C, 29 * C), name="w456k_sb")
        mbi = sb((4, 16 * C + 512), name="mbi_sb")
        mb2, ind = mbi[:, : 16 * C], mbi[:, 16 * C :]

        # PE warmup: scratch matmuls bridging to the first real matmul so
        # the HAM clock gate is warm when the stream starts.
        warm = sb((C, 512), name="warm_sb")
        nc.vector.memset(warm[:, :], 0.0)
        warm_ps = psum_pool.tile([C, PT], F32, tag="pt", name="pw")
        for _ in range(6):
            nc.tensor.matmul(warm_ps[:, 0:512], warm[:, 0:C], warm[:, :], start=True, stop=True)

        # Input DMAs ride the two HWDGE rings only (the gpsimd SWDGE ring
        # starves HWDGE under contention).  xt lands strictly first in
        # 1024-col chunks alternating rings (just-in-time for the s0/L1
        # interleave); bulk weights follow in need order.
        nc.sync.dma_start(out=wxf[:, :], in_=wxf_d[:, :])
        nc.scalar.dma_start(out=bia[:, :], in_=bia_d[:, :])
        nc.sync.dma_start(out=xt[:, 0:1024], in_=xt_d[:, 0:1024])
        nc.scalar.dma_start(out=xt[:, 1024:2048], in_=xt_d[:, 1024:2048])
        nc.sync.dma_start(out=w123[:, : 4 * C], in_=w123_d[:, : 4 * C])
        nc.scalar.dma_start(out=xt[:, 2048:3072], in_=xt_d[:, 2048:3072])
        nc.sync.dma_start(out=xt[:, 3072:4096], in_=xt_d[:, 3072:4096])
        nc.scalar.dma_start(out=xt[:, 4096:5120], in_=xt_d[:, 4096:5120])
        nc.sync.dma_start(out=xt[:, 5120:6144], in_=xt_d[:, 5120:6144])
        nc.scalar.dma_start(out=xt[:, 6144:7168], in_=xt_d[:, 6144:7168])
        nc.sync.dma_start(out=xt[:, 7168:8192], in_=xt_d[:, 7168:8192])
        nc.scalar.dma_start(out=mbi[:, :], in_=mbi_d[:, :])
        nc.sync.dma_start(out=w123[:, 4 * C : 12 * C], in_=w123_d[:, 4 * C : 12 * C])
        nc.scalar.dma_start(out=w123[:, 12 * C :], in_=w123_d[:, 12 * C :])
        nc.sync.dma_start(out=wm[:, : 32 * C], in_=wm_d[:, : 32 * C])
        nc.scalar.dma_start(out=wm[:, 32 * C :], in_=wm_d[:, 32 * C :])
        nc.sync.dma_start(out=w456k[:, :], in_=w456k_d[:, :])
        w1, w2, w3 = w123[:, : 4 * C], w123[:, 4 * C : 12 * C], w123[:, 12 * C : 28 * C]
        w4, w5 = w456k[:, : 16 * C], w456k[:, 16 * C : 24 * C]
        w6, wkf = w456k[:, 24 * C : 28 * C], w456k[:, 28 * C : 29 * C]
        xb, b1, b2 = bia[:, 0:1], bia[:, 1:3], bia[:, 3:7]
        b3, b4, b5 = bia[:, 7:15], bia[:, 15:23], bia[:, 23:27]
        b6, mb = bia[:, 27:29], bia[:, 29:93]

        load_ns = {"s": 3500.0, "v": 0.0}

        def epi(out_ap, in_ap, bias_ap, relu=True, cols=PT2):
            """One epilogue op: out = relu(in + bias) (or copy); greedy engine balance.
            V's fixed overhead is ~210ns measured (not the 120 uop model) --
            underestimating it overloads Vector by ~5us across the kernel."""
            cost = {"s": (352 + cols) / 1.2, "v": (210 + cols) / 0.96}
            eng = "s" if load_ns["s"] + cost["s"] <= load_ns["v"] + cost["v"] else "v"
            load_ns[eng] += cost[eng]
            if bias_ap is None and not relu:
                if eng == "s":
                    nc.scalar.activation(out_ap, in_ap, AF.Copy)
                else:
                    nc.vector.tensor_copy(out_ap, in_ap)
            elif bias_ap is None:
                if eng == "s":
                    nc.scalar.activation(out_ap, in_ap, AF.Relu)
                else:
                    nc.vector.tensor_scalar_max(out_ap, in_ap, 0.0)
            else:
                if eng == "s":
                    nc.scalar.activation(out_ap, in_ap, AF.Relu, bias=bias_ap)
                else:
                    nc.vector.tensor_scalar(out_ap, in_ap, bias_ap, 0.0, ALU.add, ALU.max)

        # ---------------- stage 0: input conv ----------------
        v0 = act_pool.tile([C, NCOL], BF16, tag="act", name="v0")

        def s0_tiles(bts, split=False):
            for bt in bts:
                for t in (2 * bt, 2 * bt + 1):
                    pt = psum_pool.tile([C, PT], F32, tag="pt", name="p0")
                    for s in range(2):
                        col = t * PT + s * SUB
                        nc.tensor.matmul(
                            pt[:, s * SUB : (s + 1) * SUB],
                            wxf[:, :],
                            xt[:, col : col + SUB],
                            start=True,
                            stop=True,
                        )
                    if split:
                        # halve the epilogue latency on chain-critical tiles
                        epi(v0[:, t * PT : t * PT + SUB], pt[:, 0:SUB], xb[:, 0:1], cols=SUB)
                        epi(v0[:, t * PT + SUB : (t + 1) * PT], pt[:, SUB:], xb[:, 0:1], cols=SUB)
                    else:
                        epi(v0[:, t * PT : (t + 1) * PT], pt[:, :], xb[:, 0:1], cols=PT)

        # ---------------- down levels 1..3 ----------------
        def down_level(vin, vout, w_sb, b_sb, nb_out, l_out, bts=None, split=()):
            """vin: (c, [nb_in, 2*l_out, n]); vout: (c, [nb_out, l_out, n]).
            Per 2048-col psum tile: matmuls grouped per branch (k outer, s
            inner -> same-weight runs); one 2048 epilogue when a single
            branch covers the tile, else two 1024 halves (per-branch bias)."""
            wv = w_sb.rearrange("p (b k d) -> p b k d", b=nb_out, k=2, d=C)
            vi = vin.rearrange("p (b l k n) -> p b l k n", b=nb_out // 2, l=l_out, k=2, n=NPC)
            vo = vout.rearrange("p (b l n) -> p b l n", b=nb_out, l=l_out, n=NPC)
            cpb = l_out * NPC  # columns per output branch (>= 1024)
            for bt in bts if bts is not None else range(NCOL // PT2):
                for t in (2 * bt, 2 * bt + 1):
                    pt = psum_pool.tile([C, PT], F32, tag="pt", name="pd")
                    for k in range(2):
                        for s in range(2):
                            col = t * PT + s * SUB
                            b = col // cpb
                            l0 = (col % cpb) // NPC
                            nc.tensor.matmul(
                                pt[:, s * SUB : (s + 1) * SUB],
                                wv[:, b, k, :],
                                vi[:, b // 2, l0 : l0 + SUB // NPC, k, :],
                                start=(k == 0),
                                stop=(k == 1),
                            )
                    b = (t * PT) // cpb
                    l0 = ((t * PT) % cpb) // NPC
                    if bt in split:
                        h = SUB // NPC
                        epi(vo[:, b, l0 : l0 + h, :], pt[:, 0:SUB], b_sb[:, b : b + 1], cols=SUB)
                        epi(vo[:, b, l0 + h : l0 + 2 * h, :], pt[:, SUB:], b_sb[:, b : b + 1], cols=SUB)
                    else:
                        epi(
                            vo[:, b, l0 : l0 + PT // NPC, :],
                            pt[:, :],
                            b_sb[:, b : b + 1],
                            cols=PT,
                        )

        v1 = act_pool.tile([C, NCOL], BF16, tag="act", name="v1")
        # Interleave s0 and L1 so the PE always has ready work queued behind
        # every epilogue wait: L1 bts (0,2) depend only on s0 (0,1), so L1(2)
        # fills the gap while s0(3)'s epilogue completes, and L1(3) covers
        # L1(1)'s epilogue latency before L2 starts.
        s0_tiles((0, 1))
        down_level(v0, v1, w1, b1, 2, 32, bts=(0,))
        s0_tiles((2,))
        s0_tiles((3,), split=True)
        down_level(v0, v1, w1, b1, 2, 32, bts=(2, 1, 3), split=(1,))
        v2 = act_pool.tile([C, NCOL], BF16, tag="act", name="v2")
        down_level(v1, v2, w2, b2, 4, 16)
        v3 = act_pool.tile([C, NCOL], BF16, tag="act", name="v3")
        # Every mid tile reads strided columns from ALL of v3, so the mid
        # stage gates on L3's last epilogue: split it across both engines.
        down_level(v2, v3, w3, b3, 8, 8, split=(3,))

        # ---------------- middle switch ----------------
        # v3: (c, [itk=8, itx=8, n]); vm: (c, [itx=8, itk=8, n])
        # The mid epilogue is the expensive one (per-block biases).  Split it:
        # odd tiles seed the bias into PSUM via K=4 indicator matmuls (cheap
        # on the PE) + a Scalar relu; even tiles use a Vector TT-add with a
        # broadcast bias view + an in-place DVE 4x relu.  This keeps both
        # engines fed without Vector becoming the mid-stage bottleneck.
        vm = act_pool.tile([C, NCOL], BF16, tag="act", name="vm")
        v3v = v3.rearrange("p (k x n) -> p k x n", k=8, x=8, n=NPC)
        wmv = wm.rearrange("p (k x d) -> p k x d", k=8, x=8, d=C)
        # Odd (indicator) tiles first in each pair: their bias matmuls don't
        # read v3, so they cover L3's final epilogue latency.
        def mid_tiles(ts):
          for t in ts:  # tile t covers itx = t
            pt = psum_pool.tile([C, PT], F32, tag="pt", name="pm")
            if t % 2 == 1:
                for sgrp in range(2):
                    nc.tensor.matmul(
                        pt[:, sgrp * SUB : (sgrp + 1) * SUB],
                        mb2[:, (2 * t + sgrp) * C : (2 * t + sgrp + 1) * C],
                        ind[:, :],
                        start=True,
                        stop=False,
                        skip_group_check=True,
                    )
                    for bi in range(4):
                        blk = 4 * sgrp + bi
                        nc.tensor.matmul(
                            pt[:, blk * NPC : (blk + 1) * NPC],
                            wmv[:, blk, t, :],
                            v3v[:, blk, t, :],
                            start=False,
                            stop=(bi == 3),
                            skip_group_check=True,
                        )
                nc.scalar.activation(
                    vm[:, t * PT : (t + 1) * PT], pt[:, :], AF.Relu
                )
                load_ns["s"] += (352 + PT) / 1.2
            else:
                for blk in range(8):  # block within tile (= itk)
                    nc.tensor.matmul(
                        pt[:, blk * NPC : (blk + 1) * NPC],
                        wmv[:, blk, t, :],
                        v3v[:, blk, t, :],
                        start=True,
                        stop=True,
                    )
                ptv = pt.rearrange("p (b n) -> p b n", b=8, n=NPC)
                bias_v = mb[:, 8 * t : 8 * (t + 1)].unsqueeze(2).broadcast_to((C, 8, NPC))
                dst = vm[:, t * PT : (t + 1) * PT]
                dstv = dst.rearrange("p (b n) -> p b n", b=8, n=NPC)
                nc.vector.tensor_tensor(dstv, ptv, bias_v, ALU.add)
                # in-place relu: bf16 SBUF contiguous -> DVE 4x mode, cheap
                nc.vector.tensor_scalar_max(dst, dst, 0.0)
                load_ns["v"] += (210 + PT) / 0.96 + (210 + PT / 4) / 0.96

        # ---------------- up levels 4..6 ----------------
        def up_level(vin, vout, w_sb, b_sb, nb_in, l_in, bts=None):
            """vin: (c, [x=nb_in, l_in, n]); vout: (c, [xo=nb_in/2, 2*l_in, n]);
            vout[:, xo, 2*l+j, :] = relu(sum_k vin[:, 2xo+k, l, :] @ W[xo,j,k] + B[xo,j])."""
            nbo = nb_in // 2
            wv = w_sb.rearrange("p (x j k d) -> p x j k d", x=nbo, j=2, k=2, d=C)
            vi = vin.rearrange("p (x l n) -> p x l n", x=nb_in, l=l_in, n=NPC)
            vo = vout.rearrange("p (x l j n) -> p x l j n", x=nbo, l=l_in, j=2, n=NPC)
            cpb = l_in * NPC  # columns per (xo, j) output block
            for bt in bts if bts is not None else range(NCOL // PT2):
                for t in (2 * bt, 2 * bt + 1):
                    pt = psum_pool.tile([C, PT], F32, tag="pt", name="pu")
                    for k in range(2):
                        for s in range(2):
                            col = t * PT + s * SUB
                            g = col // cpb  # (xo, j) block index, j-minor
                            xo, j = g // 2, g % 2
                            lt0 = (col % cpb) // NPC
                            nc.tensor.matmul(
                                pt[:, s * SUB : (s + 1) * SUB],
                                wv[:, xo, j, k, :],
                                vi[:, 2 * xo + k, lt0 : lt0 + SUB // NPC, :],
                                start=(k == 0),
                                stop=(k == 1),
                            )
                    g = (t * PT) // cpb
                    xo, j = g // 2, g % 2
                    lt0 = ((t * PT) % cpb) // NPC
                    epi(
                        vo[:, xo, lt0 : lt0 + PT // NPC, j, :],
                        pt[:, :],
                        b_sb[:, 2 * xo + j : 2 * xo + j + 1],
                        cols=PT,
                    )

        v4 = act_pool.tile([C, NCOL], BF16, tag="act", name="v4")
        mid_tiles((1, 0, 3, 2, 5, 4, 7, 6))
        up_level(vm, v4, w4, b4, 8, 8)
        v5 = act_pool.tile([C, NCOL], BF16, tag="act", name="v5")
        # L6 bt0/bt2 read v5 bts (0,2); emit those first so L6 can start
        # one L5 big-tile earlier.
        up_level(v4, v5, w5, b5, 4, 16, bts=(0, 2, 1, 3))
        v6 = act_pool.tile([C, NCOL], BF16, tag="act", name="v6")
        yo = singles.tile([C, NCOL], BF16, tag="yo_sb", name="yo")

        # ---------------- output conv (no bias / relu), interleaved with L6 --
        # Output leaves in 2048-col (512KB) DMAs alternating sync/scalar:
        # big enough for decent DMA bandwidth (4KB runs), small enough that
        # the final drain after the last epilogue stays short.
        out_rings = {0: nc.sync, 1: nc.scalar, 2: nc.sync, 3: nc.scalar}

        def out_tiles(bts):
            for bt in bts:
                for t in (2 * bt, 2 * bt + 1):
                    pt = psum_pool.tile([C, PT], F32, tag="pt", name="po")
                    for s in range(2):
                        col = t * PT + s * SUB
                        nc.tensor.matmul(
                            pt[:, s * SUB : (s + 1) * SUB],
                            wkf[:, :],
                            v6[:, col : col + SUB],
                            start=True,
                            stop=True,
                        )
                    if bt == 3:
                        # final tiles: 512-col epilogues on both engines so
                        # the last yo columns are ready sooner, then drain
                        # each 1024 immediately on its own ring.
                        epi(yo[:, t * PT : t * PT + SUB], pt[:, 0:SUB], None, relu=False, cols=SUB)
                        epi(yo[:, t * PT + SUB : (t + 1) * PT], pt[:, SUB:], None, relu=False, cols=SUB)
                        ring = nc.sync if t % 2 == 0 else nc.scalar
                        ring.dma_start(
                            out=out_d[:, t * PT : (t + 1) * PT],
                            in_=yo[:, t * PT : (t + 1) * PT],
                        )
                    else:
                        epi(yo[:, t * PT : (t + 1) * PT], pt[:, :], None, relu=False, cols=PT)
                if bt != 3:
                    out_rings[bt].dma_start(
                        out=out_d[:, bt * PT2 : (bt + 1) * PT2],
                        in_=yo[:, bt * PT2 : (bt + 1) * PT2],
                    )

        # L6 j=0 big-tiles are 0,1; j=1 are 2,3 (cpb=4096).  out big-tiles
        # (0,1) need L6 (0,2); out (2,3) need L6 (1,3): order so the PE has
        # L6 work queued behind every epilogue the out matmuls wait on.
        up_level(v5, v6, w6, b6, 2, 32, bts=(0, 2, 1))
        out_tiles((0,))
        up_level(v5, v6, w6, b6, 2, 32, bts=(3,))
        out_tiles((1, 2, 3))

    nc.finalize()
    return nc


_NC_CACHE = {}


def _get_nc():
    if "nc" not in _NC_CACHE:
        _NC_CACHE["nc"] = build_nc()
    return _NC_CACHE["nc"]


def _prep_in_maps(inputs):
    x = np.asarray(inputs["x"], np.float32)
    bf = lambda a: np.ascontiguousarray(np.asarray(a, np.float32)).astype(ml_dtypes.bfloat16)
    f32 = lambda a: np.ascontiguousarray(np.asarray(a, np.float32))
    mbv = np.asarray(inputs["mb"], np.float32)  # (k=8, x=8, c)
    mbT = mbv.transpose(1, 0, 2).reshape(64, C).T  # (c, 64), col = x*8 + k
    wmat = lambda key, nb: np.asarray(inputs[key], np.float32).reshape(nb, C, C).transpose(1, 0, 2).reshape(C, nb * C)
    w123 = np.concatenate([wmat("f1", 4), wmat("f2", 8), wmat("f3", 16)], axis=1)
    w456k = np.concatenate(
        [wmat("f4", 16), wmat("f5", 8), wmat("f6", 4), np.asarray(inputs["kf"], np.float32)], axis=1
    )
    bia = np.concatenate(
        [
            np.asarray(inputs["xb"], np.float32).reshape(C, 1),
            np.asarray(inputs["b1"], np.float32).T,
            np.asarray(inputs["b2"], np.float32).T,
            np.asarray(inputs["b3"], np.float32).T,
            np.asarray(inputs["b4"], np.float32).T,
            np.asarray(inputs["b5"], np.float32).T,
            np.asarray(inputs["b6"], np.float32).T,
            mbT,
        ],
        axis=1,
    )
    # mid-bias lhsT slices: u = 2*t + sgrp (t = itx tile, sgrp = 512-col half);
    # row ki covers block k = 4*sgrp + ki at x = t: mb2[ki, u*C+d] = mb[4*(u%2)+ki, u//2, d]
    mbi = np.zeros((4, 16 * C + 512), np.float32)
    for u in range(16):
        t_, sgrp = u // 2, u % 2
        for ki in range(4):
            mbi[ki, u * C : (u + 1) * C] = mbv[4 * sgrp + ki, t_, :]
    for ki in range(4):
        mbi[ki, 16 * C + ki * NPC : 16 * C + (ki + 1) * NPC] = 1.0
    shared = {
        "mbi": bf(mbi),
        "wxf": bf(inputs["xf"]),  # (f=128, c) as lhsT directly
        "w123": bf(w123),
        "wm": bf(np.asarray(inputs["md"], np.float32).reshape(64, C, C).transpose(1, 0, 2).reshape(C, 64 * C)),
        "w456k": bf(w456k),
        "bia": f32(bia),
    }
    in_maps = []
    for i in range(N_CORES):
        xs = x[i * NPC : (i + 1) * NPC]  # (128, 8192)
        xt = (
            np.ascontiguousarray(xs.reshape(NPC, 64, C).transpose(2, 1, 0))
            .reshape(C, NCOL)
            .astype(ml_dtypes.bfloat16)
        )
        in_maps.append({"xt": xt, **shared})
    return in_maps


def _gather(results):
    outs = []
    for i in range(N_CORES):
        r = np.asarray(results[i]["out"]).astype(np.float32)  # (C=k_out, [l=64, n=128])
        outs.append(r.reshape(C, 64, NPC).transpose(2, 1, 0).reshape(NPC, 64 * C))
    return np.concatenate(outs, axis=0).astype(np.float32)


def _enable_ntff_hook():
    """Register the axon NTFF profiling hook (missing from this image's
    antenv) so run_bass_kernel_spmd(trace=True) can measure HW exec time."""
    import types

    if "antenv.axon_hooks" in sys.modules:
        return
    import antenv
    from trn_agent_boot.trn_boot import _ntff_profile_via_ctypes

    hook = _ntff_profile_via_ctypes("/opt/axon/libaxon_pjrt.so")
    mod = types.ModuleType("antenv.axon_hooks")
    mod.get_axon_ntff_profile_hook = lambda: hook
    mod.set_axon_ntff_profile_hook = lambda h: None
    sys.modules["antenv.axon_hooks"] = mod
    antenv.axon_hooks = mod
    import concourse.bass_utils as bu

    bu.upload_artifacts = lambda tmpdir: tmpdir  # keep artifacts local


def run(inputs, trace=False, **kw):
    nc = _get_nc()
    in_maps = _prep_in_maps(inputs)
    if trace:
        _enable_ntff_hook()
    res = run_bass_kernel_spmd(nc, in_maps, core_ids=list(range(N_CORES)), trace=trace, **kw)
    return _gather(res.results), res


def kernel(**inputs) -> np.ndarray:
    out, _ = run(inputs, trace=False)
    return out

